# revision 49
# baseline (speedup 1.0000x reference)
# PointNet++ feature-propagation (three_nn + three_interpolate + shared MLP)
# Trainium2 Bass/Tile kernel, 8 NeuronCores, data-parallel over batch.
#
# Per batch (n=4096 unknown, m=1024 known, C2=512, C1=256):
#  1) G = W0a^T @ known_feats (256 x 1024, bf16) -- folds the MLP's
#     first-layer interp half into the gather table, halving gathered
#     channels; staged as G^T rows in SBUF for the SBUF-source DGE gather.
#  2) D = 2u.k - |k|^2 - |u|^2 = -d2 via ONE bf16 matmul with K=24 rows
#     (triple bf16 splits of u, 2k, -|k|^2, -|u|^2); MAX8/FIND_INDEX8 read
#     the fp32 PSUM directly -> top-3 neighbors AND exact-ish d2 values
#     (no refine pass, no tie-duplication).
#  3) inverse-distance weights from -top3 values (fp32 math on tiny tiles).
#  4) dma_gather (DGE, SBUF source, ~8 ns/idx) pulls G^T rows channel-major;
#     weights broadcast via ones-matmul; weighted sum on DVE at 2x (all
#     operands contiguous bf16 SBUF).
#  5) MLP1 = relu(interp + W0b^T uf) via identity-matmul PSUM inject;
#     MLP2 as usual; fp32 out.
#
# Program is software-pipelined: phase A (prep/D/selection/idx) for both
# batches is emitted before phase B (gather/wsum/MLP) so the in-order
# engine queues let batch 1's phase A overlap batch 0's gathers.
import numpy as np
from contextlib import ExitStack

import concourse.bass as bass
import concourse.bacc as bacc
import concourse.tile as tile
import concourse.mybir as mybir
from concourse.masks import make_identity

AP = bass.AP
dt = mybir.dt
Alu = mybir.AluOpType
ACTF = mybir.ActivationFunctionType

B_FULL = 16
N_CORES = 8
NB = 2            # batches per core
N = 4096
M = 1024
C1 = 256
C2 = 512
D0 = 256
D1 = 256
EPS = 1e-8

NCH = N // 128    # 32 i-chunks
MCH = M // 128    # 8 j-chunks
HALF = N // 2     # 2048
HCH = NCH // 2    # 16 chunks per half
KROWS = 24


def _bf16_split3(nc, pool, x_ap, shape):
    """bf16 (hi, lo, mid) with hi+lo+mid ~= x."""
    xh = pool.tile(list(shape), dt.bfloat16, tag="sp_h")
    xl = pool.tile(list(shape), dt.bfloat16, tag="sp_l")
    xm = pool.tile(list(shape), dt.bfloat16, tag="sp_m")
    r1 = pool.tile(list(shape), dt.float32, tag="sp_r1")
    r2 = pool.tile(list(shape), dt.float32, tag="sp_r2")
    nc.vector.tensor_copy(xh[:], x_ap)
    nc.vector.tensor_sub(r1[:], x_ap, xh[:])
    nc.vector.tensor_copy(xl[:], r1[:])
    nc.vector.tensor_sub(r2[:], r1[:], xl[:])
    nc.vector.tensor_copy(xm[:], r2[:])
    return xh, xl, xm


def _v(t_ap, dims, off=0):
    """AP over t_ap's tensor with explicit [stride, count] dims (dims[0] = partition dim)."""
    return AP(t_ap.tensor, t_ap.offset + off, dims)


def build_nc(nb=NB):
    nc = bacc.Bacc("TRN2", target_bir_lowering=False, debug=False)

    unknown_h = nc.dram_tensor("unknown", [nb, N, 3], dt.float32, kind="ExternalInput")
    known_h = nc.dram_tensor("known", [nb, M, 3], dt.float32, kind="ExternalInput")
    uf_h = nc.dram_tensor("unknow_feats", [nb, C1, N], dt.float32, kind="ExternalInput")
    kf_h = nc.dram_tensor("known_feats", [nb, C2, M], dt.float32, kind="ExternalInput")
    w0_h = nc.dram_tensor("W0", [C1 + C2, D0], dt.float32, kind="ExternalInput")
    w1_h = nc.dram_tensor("W1", [D0, D1], dt.float32, kind="ExternalInput")
    out_h = nc.dram_tensor("out", [nb, D1, N], dt.float32, kind="ExternalOutput")



    with tile.TileContext(nc) as tc, ExitStack() as ctx:
        const = ctx.enter_context(tc.tile_pool(name="const", bufs=1))
        kfp = ctx.enter_context(tc.tile_pool(name="kfp", bufs=1))
        gtp = ctx.enter_context(tc.tile_pool(name="gtp", bufs=2))
        prep = ctx.enter_context(tc.tile_pool(name="prep", bufs=2))
        sp = ctx.enter_context(tc.tile_pool(name="split", bufs=2))
        sel = ctx.enter_context(tc.tile_pool(name="sel", bufs=2))
        wrp = ctx.enter_context(tc.tile_pool(name="wrp", bufs=2))
        wr2p = ctx.enter_context(tc.tile_pool(name="wr2p", bufs=1))
        wbp = ctx.enter_context(tc.tile_pool(name="wbp", bufs=1))
        gp = ctx.enter_context(tc.tile_pool(name="gp", bufs=1))
        gwp = ctx.enter_context(tc.tile_pool(name="gwp", bufs=1))
        itp = ctx.enter_context(tc.tile_pool(name="itp", bufs=2))
        ufp = ctx.enter_context(tc.tile_pool(name="ufp", bufs=2))
        hp = ctx.enter_context(tc.tile_pool(name="hp", bufs=2))
        op = ctx.enter_context(tc.tile_pool(name="op", bufs=2))
        ps_d = ctx.enter_context(tc.tile_pool(name="ps_d", bufs=2, space="PSUM"))
        ps_mm = ctx.enter_context(tc.tile_pool(name="ps_mm", bufs=2, space="PSUM"))
        ps_tr = ctx.enter_context(tc.tile_pool(name="ps_tr", bufs=1, space="PSUM"))

        # ---------------- constants ----------------
        ident_b = const.tile([128, 128], dt.bfloat16, tag="idb")
        make_identity(nc, ident_b[:])
        ident_u = const.tile([128, 128], dt.float16, tag="idu")
        make_identity(nc, ident_u[:])
        ones_b = const.tile([1, 128], dt.bfloat16, tag="ones")
        nc.vector.memset(ones_b[:], 1.0)

        w0_sb = const.tile([128, 6, D0], dt.bfloat16, tag="w0")
        w1_sb = const.tile([128, 2, D1], dt.bfloat16, tag="w1")
        w0_f = const.tile([128, 6, D0], dt.float32, tag="w0f")
        w1_f = const.tile([128, 2, D1], dt.float32, tag="w1f")
        for ci in range(6):
            nc.sync.dma_start(w0_f[:, ci, :], w0_h.ap()[128 * ci:128 * ci + 128, :])
            nc.vector.tensor_copy(w0_sb[:, ci, :], w0_f[:, ci, :])
        for ci in range(2):
            nc.sync.dma_start(w1_f[:, ci, :], w1_h.ap()[128 * ci:128 * ci + 128, :])
            nc.vector.tensor_copy(w1_sb[:, ci, :], w1_f[:, ci, :])

        # Software pipeline: prep for ALL batches, then selection/idx for all
        # batches, then phase B (gather -> wsum -> MLP) for all batches.
        # Engine queues are in-order, so this keeps both batches' D/selection
        # back-to-back on PE/DVE while the gathers stream on GPSIMD.
        stA = []
        st = []
        for b in range(nb):
            # ======== G = W0a^T @ known_feats, staged as G^T rows in SBUF ====
            # gts[p, r, :] = G^T row (r*128 + p) = G[:, r*128+p]  (256 bf16)
            kf16 = kfp.tile([128, 4, M], dt.bfloat16, tag="kf16")
            for cj in range(4):
                kf32 = kfp.tile([128, M], dt.float32, tag="kf32")
                nc.sync.dma_start(kf32[:], kf_h.ap()[b, 128 * cj:128 * cj + 128, :])
                nc.scalar.copy(kf16[:, cj, :], kf32[:])
            gsb = gtp.tile([128, 2, M], dt.bfloat16, tag="gsb")
            for mj in range(2):
                for mh in range(2):
                    pg = ps_mm.tile([128, 512], dt.float32, tag="mm")
                    for cj in range(4):
                        nc.tensor.matmul(
                            pg[:],
                            w0_sb[:, cj, 128 * mj:128 * mj + 128],
                            kf16[:, cj, 512 * mh:512 * mh + 512],
                            start=(cj == 0),
                            stop=(cj == 3),
                        )
                    nc.scalar.copy(gsb[:, mj, 512 * mh:512 * mh + 512], pg[:])
            gts = gtp.tile([128, MCH, 2, 128], dt.bfloat16, tag="gts")
            for mj in range(2):
                pgt = ps_tr.tile([128, 1024], dt.bfloat16, tag="tr")
                for mt in range(MCH):
                    nc.tensor.transpose(
                        pgt[:, 128 * mt:128 * mt + 128],
                        gsb[:, mj, 128 * mt:128 * mt + 128],
                        ident_b[:],
                    )
                nc.scalar.copy(
                    _v(gts[:], [gts[:].ap[0], [256, MCH], [1, 128]], off=128 * mj),
                    _v(pgt[:], [pgt[:].ap[0], [128, MCH], [1, 128]]),
                )

            # ======== known prep ========
            kw = prep.tile([128, MCH, 3], dt.float32, tag="kw")
            nc.sync.dma_start(
                kw[:], AP(known_h, b * M * 3, [[3, 128], [3 * 128, MCH], [1, 3]])
            )
            k2 = prep.tile([128, MCH, 3], dt.float32, tag="k2")
            nc.vector.tensor_scalar_mul(k2[:], kw[:], 2.0)
            k2h, k2l, k2m = _bf16_split3(nc, sp, k2[:], [128, MCH, 3])
            sq = prep.tile([128, MCH, 3], dt.float32, tag="ksq")
            nc.scalar.square(sq[:], kw[:])
            s_f = prep.tile([128, MCH], dt.float32, tag="ks")
            nc.vector.tensor_add(s_f[:], sq[:, :, 0], sq[:, :, 1])
            nc.vector.tensor_add(s_f[:], s_f[:], sq[:, :, 2])
            ns = prep.tile([128, MCH], dt.float32, tag="kns")
            nc.vector.tensor_scalar_mul(ns[:], s_f[:], -1.0)
            nsh, nsl, nsm = _bf16_split3(nc, sp, ns[:], [128, MCH])

            # rows: 0-2 uh|2kh, 3-5 uh|2kl, 6-8 ul|2kh, 9-11 ul|2kl,
            #       12-14 uh|2km, 15-17 um|2kh, 18 1|-sh, 19 1|-sl, 20 1|-sm,
            #       21 -u2h|1, 22 -u2l|1, 23 -u2m|1
            kch = prep.tile([128, MCH, 24], dt.bfloat16, tag="kch")
            for (r0, src) in ((0, k2h), (3, k2l), (6, k2h), (9, k2l), (12, k2m), (15, k2h)):
                nc.vector.tensor_copy(kch[:, :, r0:r0 + 3], src[:])
            nc.vector.tensor_copy(kch[:, :, 18], nsh[:])
            nc.vector.tensor_copy(kch[:, :, 19], nsl[:])
            nc.vector.tensor_copy(kch[:, :, 20], nsm[:])
            nc.vector.memset(kch[:, :, 21:24], 1.0)
            rhs_all = prep.tile([KROWS, M], dt.bfloat16, tag="rhs_all")
            pst = ps_tr.tile([32, 1024], dt.bfloat16, tag="tr")
            for t in range(MCH):
                nc.tensor.transpose(
                    pst[:KROWS, 128 * t:128 * t + 128], kch[:, t, :KROWS], ident_b[:]
                )
            nc.scalar.copy(rhs_all[:], pst[:KROWS, :])

            # ======== unknown prep ========
            uw = prep.tile([128, NCH, 3], dt.float32, tag="uw")
            nc.sync.dma_start(
                uw[:], AP(unknown_h, b * N * 3, [[3, 128], [3 * 128, NCH], [1, 3]])
            )
            uh, ul, um = _bf16_split3(nc, sp, uw[:], [128, NCH, 3])
            usq = prep.tile([128, NCH, 3], dt.float32, tag="usq")
            nc.scalar.square(usq[:], uw[:])
            u2 = prep.tile([128, NCH], dt.float32, tag="u2")
            nc.vector.tensor_add(u2[:], usq[:, :, 0], usq[:, :, 1])
            nc.vector.tensor_add(u2[:], u2[:], usq[:, :, 2])
            nu2 = prep.tile([128, NCH], dt.float32, tag="nu2")
            nc.vector.tensor_scalar_mul(nu2[:], u2[:], -1.0)
            nu2h, nu2l, nu2m = _bf16_split3(nc, sp, nu2[:], [128, NCH])

            uch = prep.tile([128, NCH, 24], dt.bfloat16, tag="uch")
            for (r0, src) in ((0, uh), (3, uh), (6, ul), (9, ul), (12, uh), (15, um)):
                nc.vector.tensor_copy(uch[:, :, r0:r0 + 3], src[:])
            nc.vector.memset(uch[:, :, 18:21], 1.0)
            nc.vector.tensor_copy(uch[:, :, 21], nu2h[:])
            nc.vector.tensor_copy(uch[:, :, 22], nu2l[:])
            nc.vector.tensor_copy(uch[:, :, 23], nu2m[:])
            lhs_all = prep.tile([KROWS, N], dt.bfloat16, tag="lhs_all")
            for g in range(4):
                pst = ps_tr.tile([32, 1024], dt.bfloat16, tag="tr")
                for ti in range(8):
                    t = 8 * g + ti
                    nc.tensor.transpose(
                        pst[:KROWS, 128 * ti:128 * ti + 128],
                        uch[:, t, :KROWS],
                        ident_b[:],
                    )
                nc.scalar.copy(lhs_all[:, 1024 * g:1024 * g + 1024], pst[:KROWS, :])
            stA.append((gts, lhs_all, rhs_all))

        for b in range(nb):
            gts, lhs_all, rhs_all = stA[b]
            # ======== per half: D matmul + selection + weights + idx wrap ====
            # Selection and the gather-index build run per HALF so that the
            # gathers of (b, h=0) can start after only 16 chunks of selection.
            # MAX8/FIND_INDEX8 read the PSUM fp32 directly: no copy, no
            # fp16 rounding (which caused FIND_INDEX8 tie-duplicates).
            vall = sel.tile([128, NCH, 8], dt.float32, tag="vall")
            miall = sel.tile([128, NCH, 8], dt.uint16, tag="miall")
            d23 = sel.tile([128, NCH, 3], dt.float32, tag="d23")
            r3 = sel.tile([128, NCH, 3], dt.float32, tag="r3")
            z = sel.tile([128, NCH], dt.float32, tag="z")
            iz = sel.tile([128, NCH], dt.float32, tag="iz")
            w3f = sel.tile([128, NCH, 3], dt.float32, tag="w3f")
            w3b = sel.tile([128, NCH, 3], dt.bfloat16, tag="w3b")
            jf = sel.tile([128, NCH, 3], dt.float32, tag="jf")
            j3h = sel.tile([128, NCH, 3], dt.float16, tag="j3h")
            idxw = []
            for k in range(3):
                ixw_t = wrp.tile([128, N // 16], dt.int16, tag=f"idxw{k}")
                idxw.append(ixw_t)
            mitws = [[None] * 3, [None] * 3]
            for h in range(2):
                tsl = slice(HCH * h, HCH * h + HCH)
                for t in range(HCH * h, HCH * h + HCH):
                    psd = ps_d.tile([128, 1024], dt.float32, tag="psd")
                    for hm in range(2):
                        nc.tensor.matmul(
                            psd[:, 512 * hm:512 * hm + 512],
                            lhs_all[:, 128 * t:128 * t + 128],
                            rhs_all[:, 512 * hm:512 * hm + 512],
                            start=True,
                            stop=True,
                        )
                    nc.vector.max(out=vall[:, t, :], in_=psd[:])
                    nc.vector.max_index(
                        out=miall[:, t, :], in_max=vall[:, t, :], in_values=psd[:]
                    )

                # --- weights (fp32) from -d2 values: d2 = max(-v, 0) + EPS
                nc.vector.tensor_scalar(
                    d23[:, tsl, :], vall[:, tsl, 0:3], -1.0, 0.0,
                    op0=Alu.mult, op1=Alu.max,
                )
                nc.vector.tensor_scalar_add(d23[:, tsl, :], d23[:, tsl, :], EPS)
                nc.vector.reciprocal(r3[:, tsl, :], d23[:, tsl, :])
                nc.vector.tensor_reduce(
                    z[:, tsl], r3[:, tsl, :], axis=mybir.AxisListType.X, op=Alu.add
                )
                nc.vector.reciprocal(iz[:, tsl], z[:, tsl])
                nc.vector.tensor_mul(
                    w3f[:, tsl, :],
                    r3[:, tsl, :],
                    _v(iz[:], [iz[:].ap[0], [1, HCH], [0, 3]], off=HCH * h),
                )
                nc.vector.tensor_copy(w3b[:, tsl, :], w3f[:, tsl, :])
                nc.vector.tensor_copy(jf[:, tsl, :], miall[:, tsl, 0:3])
                nc.vector.tensor_copy(j3h[:, tsl, :], jf[:, tsl, :])

                # --- idx wrap + weight rows for this half
                # Token v (local point of the half) lives at [v%16, 128h + v//16]
                # of the [128, 256] int16 idx tile.
                for k in range(3):
                    ps_tj = ps_tr.tile([32, 256], dt.float16, tag="trj")
                    nc.tensor.transpose(ps_tj[:16, 0:128], j3h[:, tsl, k], ident_u[:])
                    mit = wrp.tile([16, 128], dt.float16, tag="mit")
                    nc.vector.tensor_copy(mit[:], ps_tj[:16, 0:128])
                    psx = ps_tr.tile([32, 256], dt.float16, tag="trj")
                    for s in range(8):
                        nc.tensor.transpose(
                            psx[:16, 16 * s:16 * s + 16],
                            mit[:, 16 * s:16 * s + 16],
                            ident_u[:16, :16],
                        )
                    ixw = idxw[k]
                    # ixw[q, 128h + s + 8T'] <- psx[q, 16s + T']
                    nc.vector.tensor_copy(
                        _v(ixw[:16, :], [ixw[:16, :].ap[0], [1, 8], [8, HCH]],
                           off=128 * h),
                        _v(psx[:16, :], [psx[:16, :].ap[0], [16, 8], [1, HCH]]),
                    )
                    csl = slice(128 * h, 128 * h + 128)
                    for gsz in (16, 32, 64):
                        nc.sync.dma_start(ixw[gsz:2 * gsz, csl], ixw[0:gsz, csl])
                    # --- weight row mitw[T', c] = w_k[half-local 128T' + c]
                    ps_tw = ps_tr.tile([128, 1024], dt.bfloat16, tag="tr")
                    nc.tensor.transpose(ps_tw[:16, 0:128], w3b[:, tsl, k], ident_b[:])
                    mitw = wrp.tile([16, 128], dt.bfloat16, tag=f"mitw_{k}{h}")
                    nc.scalar.copy(mitw[:], ps_tw[:16, 0:128])
                    mitws[h][k] = mitw
            st.append((gts, idxw, mitws))

        for b in range(nb):
            gts, idxw, mitws = st[b]
            # ======== per half: gather, wsum, MLP ========
            for h in range(2):
                # --- wb_k build emitted BEFORE gather k so the weighted sum
                # can fire the moment each gather lands; gathers (DGE, SBUF
                # source) land channel-major [128, 2, HALF].
                g_ts = []
                wbs = []
                for k in range(3):
                    wrow = wr2p.tile([1, HALF], dt.bfloat16, tag=f"wrow_{k % 2}")
                    nc.sync.dma_start(
                        _v(wrow[:], [wrow[:].ap[0], [128, HCH], [1, 128]]),
                        mitws[h][k][:],
                    )
                    wb = wbp.tile([128, HALF], dt.bfloat16, tag=f"wb{k % 2}")
                    for nci in range(HALF // 512):
                        ps_wb = ps_mm.tile([128, 512], dt.float32, tag="mm")
                        nc.tensor.matmul(
                            ps_wb[:],
                            ones_b[:],
                            wrow[0:1, 512 * nci:512 * nci + 512],
                            start=True,
                            stop=True,
                        )
                        nc.scalar.copy(wb[:, 512 * nci:512 * nci + 512], ps_wb[:])
                    wbs.append(wb)
                    g_t = gp.tile([128, 2, HALF], dt.bfloat16, tag=f"g{k}")
                    nc.gpsimd.dma_gather(
                        g_t[:],
                        gts[:],
                        idxw[k][:, 128 * h:128 * h + 128],
                        HALF,
                        HALF,
                        256,
                        transpose=True,
                        single_packet=False,
                        sbuf_tokens_per_rank=128,
                        sbuf_free_dim_per_rank=512,
                    )
                    g_ts.append(g_t)

                # --- interp = sum_k wb_k * g_k
                interp = itp.tile([128, 2, HALF], dt.bfloat16, tag="interp")
                for k in range(3):
                    for e in range(2):
                        if k == 0:
                            nc.vector.tensor_mul(
                                interp[:, e, :], g_ts[0][:, e, :], wbs[0][:]
                            )
                        else:
                            gwk = gwp.tile([128, HALF], dt.bfloat16, tag="gw")
                            nc.vector.tensor_mul(gwk[:], g_ts[k][:, e, :], wbs[k][:])
                            nc.vector.tensor_add(
                                interp[:, e, :], interp[:, e, :], gwk[:]
                            )

                # --- unknow_feats -> bf16
                uf16 = ufp.tile([128, 2, HALF], dt.bfloat16, tag="uf16")
                for cj in range(2):
                    for q in range(2):
                        uf32 = ufp.tile([128, 1024], dt.float32, tag="uf32")
                        nc.sync.dma_start(
                            uf32[:],
                            uf_h.ap()[
                                b, 128 * cj:128 * cj + 128,
                                HALF * h + 1024 * q:HALF * h + 1024 * q + 1024,
                            ],
                        )
                        nc.scalar.copy(uf16[:, cj, 1024 * q:1024 * q + 1024], uf32[:])

                # --- MLP1: h = relu(interp + W0b^T uf)
                h_t = hp.tile([128, 2, HALF], dt.bfloat16, tag="h")
                for mj in range(2):
                    for nci in range(HALF // 512):
                        nsl_ = slice(512 * nci, 512 * nci + 512)
                        pm = ps_mm.tile([128, 512], dt.float32, tag="mm")
                        nc.tensor.matmul(
                            pm[:],
                            ident_b[:],
                            interp[:, mj, nsl_],
                            start=True,
                            stop=False,
                        )
                        for ci in range(2):
                            nc.tensor.matmul(
                                pm[:],
                                w0_sb[:, 4 + ci, 128 * mj:128 * mj + 128],
                                uf16[:, ci, nsl_],
                                start=False,
                                stop=(ci == 1),
                            )
                        nc.scalar.activation(h_t[:, mj, nsl_], pm[:], ACTF.Relu, bias=0.0)

                # --- MLP2 (relu) -> fp32 out
                for mj in range(2):
                    for oq in range(4):
                        o_t = op.tile([128, 512], dt.float32, tag="o")
                        nsl_ = slice(512 * oq, 512 * oq + 512)
                        pm = ps_mm.tile([128, 512], dt.float32, tag="mm")
                        for ci in range(2):
                            nc.tensor.matmul(
                                pm[:],
                                w1_sb[:, ci, 128 * mj:128 * mj + 128],
                                h_t[:, ci, nsl_],
                                start=(ci == 0),
                                stop=(ci == 1),
                            )
                        nc.scalar.activation(o_t[:], pm[:], ACTF.Relu, bias=0.0)
                        nc.sync.dma_start(
                            out_h.ap()[
                                b, 128 * mj:128 * mj + 128,
                                HALF * h + 512 * oq:HALF * h + 512 * oq + 512,
                            ],
                            o_t[:],
                        )

    nc.compile()
    return nc


_NC_CACHE = {}


def _get_nc(nb=NB):
    if nb not in _NC_CACHE:
        _NC_CACHE[nb] = build_nc(nb)
    return _NC_CACHE[nb]


def kernel(**inputs):
    from concourse.bass_utils import run_bass_kernel_spmd

    nc = _get_nc(NB)
    per_core = B_FULL // N_CORES
    in_maps = []
    for c in range(N_CORES):
        sl = slice(per_core * c, per_core * (c + 1))
        in_maps.append(
            {
                "unknown": np.ascontiguousarray(np.asarray(inputs["unknown"][sl], dtype=np.float32)),
                "known": np.ascontiguousarray(np.asarray(inputs["known"][sl], dtype=np.float32)),
                "unknow_feats": np.ascontiguousarray(np.asarray(inputs["unknow_feats"][sl], dtype=np.float32)),
                "known_feats": np.ascontiguousarray(np.asarray(inputs["known_feats"][sl], dtype=np.float32)),
                "W0": np.asarray(inputs["W0"], dtype=np.float32),
                "W1": np.asarray(inputs["W1"], dtype=np.float32),
            }
        )
    res = run_bass_kernel_spmd(nc, in_maps, core_ids=list(range(N_CORES)))
    out = np.concatenate([res.results[c]["out"] for c in range(N_CORES)], axis=0)
    return out.astype(np.float32)


# revision 52
# speedup vs baseline: 1.0881x; 1.0881x over previous
# PointNet++ feature-propagation (three_nn + three_interpolate + shared MLP)
# Trainium2 Bass/Tile kernel, 8 NeuronCores, data-parallel over batch.
#
# Per batch (n=4096 unknown, m=1024 known, C2=512, C1=256):
#  1) G = W0a^T @ known_feats (256 x 1024, bf16) -- folds the MLP's
#     first-layer interp half into the gather table, halving gathered
#     channels; staged as G^T rows in SBUF for the SBUF-source DGE gather.
#  2) D = 2u.k - |k|^2 - |u|^2 = -d2 via ONE bf16 matmul with K=24 rows
#     (triple bf16 splits of u, 2k, -|k|^2, -|u|^2); MAX8/FIND_INDEX8 read
#     the fp32 PSUM directly -> top-3 neighbors AND exact-ish d2 values
#     (no refine pass, no tie-duplication).
#  3) inverse-distance weights from -top3 values (fp32 math on tiny tiles).
#  4) dma_gather (DGE, SBUF source, ~8 ns/idx) pulls G^T rows channel-major;
#     weights broadcast via ones-matmul; weighted sum on DVE at 2x (all
#     operands contiguous bf16 SBUF).
#  5) MLP1 = relu(interp + W0b^T uf) via identity-matmul PSUM inject;
#     MLP2 as usual; fp32 out.
#
# Program is software-pipelined: phase A (prep/D/selection/idx) for both
# batches is emitted before phase B (gather/wsum/MLP) so the in-order
# engine queues let batch 1's phase A overlap batch 0's gathers.
import numpy as np
from contextlib import ExitStack

import concourse.bass as bass
import concourse.bacc as bacc
import concourse.tile as tile
import concourse.mybir as mybir
from concourse.masks import make_identity

AP = bass.AP
dt = mybir.dt
Alu = mybir.AluOpType
ACTF = mybir.ActivationFunctionType

B_FULL = 16
N_CORES = 8
NB = 2            # batches per core
N = 4096
M = 1024
C1 = 256
C2 = 512
D0 = 256
D1 = 256
EPS = 1e-8

NCH = N // 128    # 32 i-chunks
MCH = M // 128    # 8 j-chunks
HALF = N // 2     # 2048
HCH = NCH // 2    # 16 chunks per half
KROWS = 24


def _bf16_split3(nc, pool, x_ap, shape):
    """bf16 (hi, lo, mid) with hi+lo+mid ~= x."""
    xh = pool.tile(list(shape), dt.bfloat16, tag="sp_h")
    xl = pool.tile(list(shape), dt.bfloat16, tag="sp_l")
    xm = pool.tile(list(shape), dt.bfloat16, tag="sp_m")
    r1 = pool.tile(list(shape), dt.float32, tag="sp_r1")
    r2 = pool.tile(list(shape), dt.float32, tag="sp_r2")
    nc.vector.tensor_copy(xh[:], x_ap)
    nc.vector.tensor_sub(r1[:], x_ap, xh[:])
    nc.vector.tensor_copy(xl[:], r1[:])
    nc.vector.tensor_sub(r2[:], r1[:], xl[:])
    nc.vector.tensor_copy(xm[:], r2[:])
    return xh, xl, xm


def _v(t_ap, dims, off=0):
    """AP over t_ap's tensor with explicit [stride, count] dims (dims[0] = partition dim)."""
    return AP(t_ap.tensor, t_ap.offset + off, dims)


def build_nc(nb=NB):
    nc = bacc.Bacc("TRN2", target_bir_lowering=False, debug=False)

    unknown_h = nc.dram_tensor("unknown", [nb, N, 3], dt.float32, kind="ExternalInput")
    known_h = nc.dram_tensor("known", [nb, M, 3], dt.float32, kind="ExternalInput")
    uf_h = nc.dram_tensor("unknow_feats", [nb, C1, N], dt.float32, kind="ExternalInput")
    kf_h = nc.dram_tensor("known_feats", [nb, C2, M], dt.float32, kind="ExternalInput")
    w0_h = nc.dram_tensor("W0", [C1 + C2, D0], dt.float32, kind="ExternalInput")
    w1_h = nc.dram_tensor("W1", [D0, D1], dt.float32, kind="ExternalInput")
    out_h = nc.dram_tensor("out", [nb, D1, N], dt.float32, kind="ExternalOutput")



    with tile.TileContext(nc) as tc, ExitStack() as ctx:
        const = ctx.enter_context(tc.tile_pool(name="const", bufs=1))
        kfp = ctx.enter_context(tc.tile_pool(name="kfp", bufs=1))
        gtp = ctx.enter_context(tc.tile_pool(name="gtp", bufs=2))
        prep = ctx.enter_context(tc.tile_pool(name="prep", bufs=2))
        sp = ctx.enter_context(tc.tile_pool(name="split", bufs=2))
        sel = ctx.enter_context(tc.tile_pool(name="sel", bufs=2))
        wrp = ctx.enter_context(tc.tile_pool(name="wrp", bufs=2))
        wr2p = ctx.enter_context(tc.tile_pool(name="wr2p", bufs=1))
        wbp = ctx.enter_context(tc.tile_pool(name="wbp", bufs=1))
        gp = ctx.enter_context(tc.tile_pool(name="gp", bufs=1))
        gwp = ctx.enter_context(tc.tile_pool(name="gwp", bufs=1))
        itp = ctx.enter_context(tc.tile_pool(name="itp", bufs=2))
        ufp = ctx.enter_context(tc.tile_pool(name="ufp", bufs=2))
        hp = ctx.enter_context(tc.tile_pool(name="hp", bufs=2))
        op = ctx.enter_context(tc.tile_pool(name="op", bufs=2))
        ps_d = ctx.enter_context(tc.tile_pool(name="ps_d", bufs=2, space="PSUM"))
        ps_mm = ctx.enter_context(tc.tile_pool(name="ps_mm", bufs=2, space="PSUM"))
        ps_tr = ctx.enter_context(tc.tile_pool(name="ps_tr", bufs=1, space="PSUM"))

        # ---------------- constants ----------------
        ident_b = const.tile([128, 128], dt.bfloat16, tag="idb")
        make_identity(nc, ident_b[:])
        ident_u = const.tile([128, 128], dt.float16, tag="idu")
        make_identity(nc, ident_u[:])
        ones_b = const.tile([1, 128], dt.bfloat16, tag="ones")
        nc.vector.memset(ones_b[:], 1.0)

        w0_sb = const.tile([128, 6, D0], dt.bfloat16, tag="w0")
        w1_sb = const.tile([128, 2, D1], dt.bfloat16, tag="w1")
        w0_f = const.tile([128, 6, D0], dt.float32, tag="w0f")
        w1_f = const.tile([128, 2, D1], dt.float32, tag="w1f")
        for ci in range(6):
            nc.sync.dma_start(w0_f[:, ci, :], w0_h.ap()[128 * ci:128 * ci + 128, :])
            nc.vector.tensor_copy(w0_sb[:, ci, :], w0_f[:, ci, :])
        for ci in range(2):
            nc.sync.dma_start(w1_f[:, ci, :], w1_h.ap()[128 * ci:128 * ci + 128, :])
            nc.vector.tensor_copy(w1_sb[:, ci, :], w1_f[:, ci, :])

        # Software pipeline: prep for ALL batches, then selection/idx for all
        # batches, then phase B (gather -> wsum -> MLP) for all batches.
        # Engine queues are in-order, so this keeps both batches' D/selection
        # back-to-back on PE/DVE while the gathers stream on GPSIMD.
        stA = []
        st = []
        for b in range(nb):
            # ======== G = W0a^T @ known_feats, staged as G^T rows in SBUF ====
            # gts[p, r, :] = G^T row (r*128 + p) = G[:, r*128+p]  (256 bf16)
            kf16 = kfp.tile([128, 4, M], dt.bfloat16, tag="kf16")
            for cj in range(4):
                kf32 = kfp.tile([128, M], dt.float32, tag="kf32")
                nc.sync.dma_start(kf32[:], kf_h.ap()[b, 128 * cj:128 * cj + 128, :])
                nc.scalar.copy(kf16[:, cj, :], kf32[:])
            gsb = gtp.tile([128, 2, M], dt.bfloat16, tag="gsb")
            for mj in range(2):
                for mh in range(2):
                    pg = ps_mm.tile([128, 512], dt.float32, tag="mm")
                    for cj in range(4):
                        nc.tensor.matmul(
                            pg[:],
                            w0_sb[:, cj, 128 * mj:128 * mj + 128],
                            kf16[:, cj, 512 * mh:512 * mh + 512],
                            start=(cj == 0),
                            stop=(cj == 3),
                        )
                    nc.scalar.copy(gsb[:, mj, 512 * mh:512 * mh + 512], pg[:])
            gts = gtp.tile([128, MCH, 2, 128], dt.bfloat16, tag="gts")
            for mj in range(2):
                pgt = ps_tr.tile([128, 1024], dt.bfloat16, tag="tr")
                for mt in range(MCH):
                    nc.tensor.transpose(
                        pgt[:, 128 * mt:128 * mt + 128],
                        gsb[:, mj, 128 * mt:128 * mt + 128],
                        ident_b[:],
                    )
                nc.scalar.copy(
                    _v(gts[:], [gts[:].ap[0], [256, MCH], [1, 128]], off=128 * mj),
                    _v(pgt[:], [pgt[:].ap[0], [128, MCH], [1, 128]]),
                )

            # ======== known prep ========
            kw = prep.tile([128, MCH, 3], dt.float32, tag="kw")
            nc.sync.dma_start(
                kw[:], AP(known_h, b * M * 3, [[3, 128], [3 * 128, MCH], [1, 3]])
            )
            k2 = prep.tile([128, MCH, 3], dt.float32, tag="k2")
            nc.vector.tensor_scalar_mul(k2[:], kw[:], 2.0)
            k2h, k2l, k2m = _bf16_split3(nc, sp, k2[:], [128, MCH, 3])
            sq = prep.tile([128, MCH, 3], dt.float32, tag="ksq")
            nc.scalar.square(sq[:], kw[:])
            s_f = prep.tile([128, MCH], dt.float32, tag="ks")
            nc.vector.tensor_add(s_f[:], sq[:, :, 0], sq[:, :, 1])
            nc.vector.tensor_add(s_f[:], s_f[:], sq[:, :, 2])
            ns = prep.tile([128, MCH], dt.float32, tag="kns")
            nc.vector.tensor_scalar_mul(ns[:], s_f[:], -1.0)
            nsh, nsl, nsm = _bf16_split3(nc, sp, ns[:], [128, MCH])

            # rows: 0-2 uh|2kh, 3-5 uh|2kl, 6-8 ul|2kh, 9-11 ul|2kl,
            #       12-14 uh|2km, 15-17 um|2kh, 18 1|-sh, 19 1|-sl, 20 1|-sm,
            #       21 -u2h|1, 22 -u2l|1, 23 -u2m|1
            kch = prep.tile([128, MCH, 24], dt.bfloat16, tag="kch")
            for (r0, src) in ((0, k2h), (3, k2l), (6, k2h), (9, k2l), (12, k2m), (15, k2h)):
                nc.vector.tensor_copy(kch[:, :, r0:r0 + 3], src[:])
            nc.vector.tensor_copy(kch[:, :, 18], nsh[:])
            nc.vector.tensor_copy(kch[:, :, 19], nsl[:])
            nc.vector.tensor_copy(kch[:, :, 20], nsm[:])
            nc.vector.memset(kch[:, :, 21:24], 1.0)
            rhs_all = prep.tile([KROWS, M], dt.bfloat16, tag="rhs_all")
            pst = ps_tr.tile([32, 1024], dt.bfloat16, tag="tr")
            for t in range(MCH):
                nc.tensor.transpose(
                    pst[:KROWS, 128 * t:128 * t + 128], kch[:, t, :KROWS], ident_b[:]
                )
            nc.scalar.copy(rhs_all[:], pst[:KROWS, :])

            # ======== unknown prep ========
            uw = prep.tile([128, NCH, 3], dt.float32, tag="uw")
            nc.sync.dma_start(
                uw[:], AP(unknown_h, b * N * 3, [[3, 128], [3 * 128, NCH], [1, 3]])
            )
            uh, ul, um = _bf16_split3(nc, sp, uw[:], [128, NCH, 3])
            usq = prep.tile([128, NCH, 3], dt.float32, tag="usq")
            nc.scalar.square(usq[:], uw[:])
            u2 = prep.tile([128, NCH], dt.float32, tag="u2")
            nc.vector.tensor_add(u2[:], usq[:, :, 0], usq[:, :, 1])
            nc.vector.tensor_add(u2[:], u2[:], usq[:, :, 2])
            nu2 = prep.tile([128, NCH], dt.float32, tag="nu2")
            nc.vector.tensor_scalar_mul(nu2[:], u2[:], -1.0)
            nu2h, nu2l, nu2m = _bf16_split3(nc, sp, nu2[:], [128, NCH])

            uch = prep.tile([128, NCH, 24], dt.bfloat16, tag="uch")
            for (r0, src) in ((0, uh), (3, uh), (6, ul), (9, ul), (12, uh), (15, um)):
                nc.vector.tensor_copy(uch[:, :, r0:r0 + 3], src[:])
            nc.vector.memset(uch[:, :, 18:21], 1.0)
            nc.vector.tensor_copy(uch[:, :, 21], nu2h[:])
            nc.vector.tensor_copy(uch[:, :, 22], nu2l[:])
            nc.vector.tensor_copy(uch[:, :, 23], nu2m[:])
            lhs_all = prep.tile([KROWS, N], dt.bfloat16, tag="lhs_all")
            for g in range(4):
                pst = ps_tr.tile([32, 1024], dt.bfloat16, tag="tr")
                for ti in range(8):
                    t = 8 * g + ti
                    nc.tensor.transpose(
                        pst[:KROWS, 128 * ti:128 * ti + 128],
                        uch[:, t, :KROWS],
                        ident_b[:],
                    )
                nc.scalar.copy(lhs_all[:, 1024 * g:1024 * g + 1024], pst[:KROWS, :])

            # ======== D matmul (-d2) + top-8 selection per i-chunk ========
            # MAX8/FIND_INDEX8 read the PSUM fp32 directly: no copy, no
            # fp16 rounding (which caused FIND_INDEX8 tie-duplicates).
            vall = sel.tile([128, NCH, 8], dt.float32, tag="vall")
            miall = sel.tile([128, NCH, 8], dt.uint16, tag="miall")
            for t in range(NCH):
                psd = ps_d.tile([128, 1024], dt.float32, tag="psd")
                for hm in range(2):
                    nc.tensor.matmul(
                        psd[:, 512 * hm:512 * hm + 512],
                        lhs_all[:, 128 * t:128 * t + 128],
                        rhs_all[:, 512 * hm:512 * hm + 512],
                        start=True,
                        stop=True,
                    )
                nc.vector.max(out=vall[:, t, :], in_=psd[:])
                nc.vector.max_index(
                    out=miall[:, t, :], in_max=vall[:, t, :], in_values=psd[:]
                )

            # ======== weights (fp32) from -d2 values ========
            d23 = sel.tile([128, NCH, 3], dt.float32, tag="d23")
            # d2 = max(-v, 0) + EPS
            nc.vector.tensor_scalar(
                d23[:], vall[:, :, 0:3], -1.0, 0.0, op0=Alu.mult, op1=Alu.max
            )
            nc.vector.tensor_scalar_add(d23[:], d23[:], EPS)
            r3 = sel.tile([128, NCH, 3], dt.float32, tag="r3")
            nc.vector.reciprocal(r3[:], d23[:])
            z = sel.tile([128, NCH], dt.float32, tag="z")
            nc.vector.tensor_reduce(z[:], r3[:], axis=mybir.AxisListType.X, op=Alu.add)
            iz = sel.tile([128, NCH], dt.float32, tag="iz")
            nc.vector.reciprocal(iz[:], z[:])
            w3f = sel.tile([128, NCH, 3], dt.float32, tag="w3f")
            nc.vector.tensor_mul(w3f[:], r3[:], iz[:].to_broadcast([128, NCH, 3]))
            w3b = sel.tile([128, NCH, 3], dt.bfloat16, tag="w3b")
            nc.vector.tensor_copy(w3b[:], w3f[:])
            jf = sel.tile([128, NCH, 3], dt.float32, tag="jf")
            nc.vector.tensor_copy(jf[:], miall[:, :, 0:3])
            j3h = sel.tile([128, NCH, 3], dt.float16, tag="j3h")
            nc.vector.tensor_copy(j3h[:], jf[:])

            # ======== per batch: idx wrap + weight rows ========
            # Token v (point i of the full batch) lives at [v%16, v//16] of a
            # [128, 256] int16 idx tile; a half's tokens are columns
            # [128h, 128h+128) of that tile.
            idxw = []
            mitws = [[None] * 3, [None] * 3]
            for k in range(3):
                # --- idx: ixw[q, 8T + s] = j3h[16s + q, T, k]
                ps_tj = ps_tr.tile([32, 256], dt.float16, tag="trj")
                nc.tensor.transpose(ps_tj[:, 0:128], j3h[:, :, k], ident_u[:])
                mit = wrp.tile([32, 128], dt.float16, tag="mit")
                nc.vector.tensor_copy(mit[:], ps_tj[:, 0:128])
                psx = ps_tr.tile([32, 256], dt.float16, tag="trj")
                for s in range(8):
                    nc.tensor.transpose(
                        psx[:16, 32 * s:32 * s + 32],
                        mit[:, 16 * s:16 * s + 16],
                        ident_u[:32, :32],
                    )
                ixw = wrp.tile([128, N // 16], dt.int16, tag=f"idxw{k}")
                # ixw[q, s + 8T] <- psx[q, 32s + T]
                nc.vector.tensor_copy(
                    _v(ixw[:16, :], [ixw[:16, :].ap[0], [1, 8], [8, NCH]]),
                    _v(psx[:16, :], [psx[:16, :].ap[0], [32, 8], [1, NCH]]),
                )
                for gsz in (16, 32, 64):
                    nc.sync.dma_start(ixw[gsz:2 * gsz, :], ixw[0:gsz, :])
                idxw.append(ixw)
                # --- weights: mitw[T, c] = w_k[128T + c]
                ps_tw = ps_tr.tile([128, 1024], dt.bfloat16, tag="tr")
                nc.tensor.transpose(ps_tw[:32, 0:128], w3b[:, :, k], ident_b[:])
                mitw = wrp.tile([32, 128], dt.bfloat16, tag=f"mitw_{k}")
                nc.scalar.copy(mitw[:], ps_tw[:32, 0:128])
                mitws[0][k] = mitw
                mitws[1][k] = mitw
            st.append((gts, idxw, mitws))

        for b in range(nb):
            gts, idxw, mitws = st[b]
            # ======== per half: gather, wsum, MLP ========
            for h in range(2):
                # --- wb_k build emitted BEFORE gather k so the weighted sum
                # can fire the moment each gather lands; gathers (DGE, SBUF
                # source) land channel-major [128, 2, HALF].
                g_ts = []
                wbs = []
                for k in range(3):
                    wrow = wr2p.tile([1, HALF], dt.bfloat16, tag=f"wrow_{k % 2}")
                    nc.sync.dma_start(
                        _v(wrow[:], [wrow[:].ap[0], [128, HCH], [1, 128]]),
                        mitws[h][k][HCH * h:HCH * h + HCH, :],
                    )
                    wb = wbp.tile([128, HALF], dt.bfloat16, tag=f"wb{k % 2}")
                    for nci in range(HALF // 512):
                        ps_wb = ps_mm.tile([128, 512], dt.float32, tag="mm")
                        nc.tensor.matmul(
                            ps_wb[:],
                            ones_b[:],
                            wrow[0:1, 512 * nci:512 * nci + 512],
                            start=True,
                            stop=True,
                        )
                        nc.scalar.copy(wb[:, 512 * nci:512 * nci + 512], ps_wb[:])
                    wbs.append(wb)
                    g_t = gp.tile([128, 2, HALF], dt.bfloat16, tag=f"g{k}")
                    nc.gpsimd.dma_gather(
                        g_t[:],
                        gts[:],
                        idxw[k][:, 128 * h:128 * h + 128],
                        HALF,
                        HALF,
                        256,
                        transpose=True,
                        single_packet=False,
                        sbuf_tokens_per_rank=128,
                        sbuf_free_dim_per_rank=512,
                    )
                    g_ts.append(g_t)

                # --- interp = sum_k wb_k * g_k
                interp = itp.tile([128, 2, HALF], dt.bfloat16, tag="interp")
                for k in range(3):
                    for e in range(2):
                        if k == 0:
                            nc.vector.tensor_mul(
                                interp[:, e, :], g_ts[0][:, e, :], wbs[0][:]
                            )
                        else:
                            gwk = gwp.tile([128, HALF], dt.bfloat16, tag="gw")
                            nc.vector.tensor_mul(gwk[:], g_ts[k][:, e, :], wbs[k][:])
                            nc.vector.tensor_add(
                                interp[:, e, :], interp[:, e, :], gwk[:]
                            )

                # --- unknow_feats -> bf16
                uf16 = ufp.tile([128, 2, HALF], dt.bfloat16, tag="uf16")
                for cj in range(2):
                    for q in range(2):
                        uf32 = ufp.tile([128, 1024], dt.float32, tag="uf32")
                        nc.sync.dma_start(
                            uf32[:],
                            uf_h.ap()[
                                b, 128 * cj:128 * cj + 128,
                                HALF * h + 1024 * q:HALF * h + 1024 * q + 1024,
                            ],
                        )
                        nc.scalar.copy(uf16[:, cj, 1024 * q:1024 * q + 1024], uf32[:])

                # --- MLP1: h = relu(interp + W0b^T uf)
                h_t = hp.tile([128, 2, HALF], dt.bfloat16, tag="h")
                for mj in range(2):
                    for nci in range(HALF // 512):
                        nsl_ = slice(512 * nci, 512 * nci + 512)
                        pm = ps_mm.tile([128, 512], dt.float32, tag="mm")
                        nc.tensor.matmul(
                            pm[:],
                            ident_b[:],
                            interp[:, mj, nsl_],
                            start=True,
                            stop=False,
                        )
                        for ci in range(2):
                            nc.tensor.matmul(
                                pm[:],
                                w0_sb[:, 4 + ci, 128 * mj:128 * mj + 128],
                                uf16[:, ci, nsl_],
                                start=False,
                                stop=(ci == 1),
                            )
                        nc.scalar.activation(h_t[:, mj, nsl_], pm[:], ACTF.Relu, bias=0.0)

                # --- MLP2 (relu) -> fp32 out
                for mj in range(2):
                    for oq in range(4):
                        o_t = op.tile([128, 512], dt.float32, tag="o")
                        nsl_ = slice(512 * oq, 512 * oq + 512)
                        pm = ps_mm.tile([128, 512], dt.float32, tag="mm")
                        for ci in range(2):
                            nc.tensor.matmul(
                                pm[:],
                                w1_sb[:, ci, 128 * mj:128 * mj + 128],
                                h_t[:, ci, nsl_],
                                start=(ci == 0),
                                stop=(ci == 1),
                            )
                        nc.scalar.activation(o_t[:], pm[:], ACTF.Relu, bias=0.0)
                        nc.sync.dma_start(
                            out_h.ap()[
                                b, 128 * mj:128 * mj + 128,
                                HALF * h + 512 * oq:HALF * h + 512 * oq + 512,
                            ],
                            o_t[:],
                        )

    nc.compile()
    return nc


_NC_CACHE = {}


def _get_nc(nb=NB):
    if nb not in _NC_CACHE:
        _NC_CACHE[nb] = build_nc(nb)
    return _NC_CACHE[nb]


def kernel(**inputs):
    from concourse.bass_utils import run_bass_kernel_spmd

    nc = _get_nc(NB)
    per_core = B_FULL // N_CORES
    in_maps = []
    for c in range(N_CORES):
        sl = slice(per_core * c, per_core * (c + 1))
        in_maps.append(
            {
                "unknown": np.ascontiguousarray(np.asarray(inputs["unknown"][sl], dtype=np.float32)),
                "known": np.ascontiguousarray(np.asarray(inputs["known"][sl], dtype=np.float32)),
                "unknow_feats": np.ascontiguousarray(np.asarray(inputs["unknow_feats"][sl], dtype=np.float32)),
                "known_feats": np.ascontiguousarray(np.asarray(inputs["known_feats"][sl], dtype=np.float32)),
                "W0": np.asarray(inputs["W0"], dtype=np.float32),
                "W1": np.asarray(inputs["W1"], dtype=np.float32),
            }
        )
    res = run_bass_kernel_spmd(nc, in_maps, core_ids=list(range(N_CORES)))
    out = np.concatenate([res.results[c]["out"] for c in range(N_CORES)], axis=0)
    return out.astype(np.float32)
